# revision 14
# baseline (speedup 1.0000x reference)
"""Trainium2 Bass kernel for nn_LocalAttentionBlock (MQA local attention, window=1024).

Sharding: 8 cores = 2 batches x 4 time-chunks of 1024 queries. Window=1024 means
each 1024-query chunk only needs the 2048 preceding tokens of x for K/V -> no
collectives; each core computes its output rows independently.

v2 (all-bf16): every matmul operand is bf16 (f32 PSUM accumulate), which
- runs transposes at 1.0 cyc/row (vs 1.5 f32r) and avoids the f32r 4x penalty
  on narrow (<256 col) chunks,
- halves DMA traffic (weights/x pre-converted to bf16 on host, contiguous
  per-tile layouts so every DMA is a plain block copy),
- enables the DVE 2x 16-bit mode for RoPE/masks/copies,
- keeps encT resident in SBUF between attention and the final projection
  (baseline bounced 16MB through DRAM).
Numerics checked against the f32 reference: max rel err ~4e-3 (budget 2e-2).

Per-core layout:
  - xT bf16 [w, t] via PE transposes; xT_lo freed after K/V proj.
  - kT = RoPE(Wk.T @ xT) bf16 [128, 2048]; v_aug bf16 [s,130] with ones col
    (PV matmul accumulates numerator AND denominator).
  - per head: qT = RoPE(Wq_h.T @ xT_hi) bf16; logits computed transposed [s, q]
    with stationary kT s-block; softmax without max-subtraction; band mask
    applied multiplicatively post-exp on the two partial diagonal blocks.
  - zero-padded history of chunk 0 contributes exp(0)=1 per padded in-band key;
    corrected by subtracting a host-computed count from the denominator.
  - encoded scaled by 1/den -> bf16, PE-transposed into SBUF-resident encT;
    final projection streams prefetched bf16 Wf tiles against encT.
"""

import numpy as np
import ml_dtypes

import concourse.bass as bass
from concourse import bacc
import concourse.mybir as mybir
import concourse.tile as tile
from concourse.bass_utils import run_bass_kernel_spmd
from concourse.masks import make_identity

F32 = mybir.dt.float32
BF16 = mybir.dt.bfloat16

B, T, W, NH, HD, WIN = 2, 4096, 2048, 16, 128, 1024
TQ, TKV = 1024, 2048
NQT = TQ // 128          # 8 query tiles
NST = TKV // 128         # 16 key tiles
NKT = W // 128           # 16 contraction tiles over width
SCALE = float(HD) ** -0.5
NB = 9                   # band blocks per query tile


def _rope(nc, dst, cs, sn, rope_p):
    """In-place partial RoPE on bf16 tile slice dst [128, n] (rows 0:64 rotated,
    64:128 untouched). cs/sn are [64, n] bf16 SBUF slices (cos/sin duplicated to
    64 rows so both TensorTensor inputs are partition-aligned; the BIR verifier
    rejects SBUF x SBUF inputs with different partition offsets)."""
    n = dst.shape[-1]
    t1 = rope_p.tile([32, n], BF16, tag="t1")
    t2 = rope_p.tile([32, n], BF16, tag="t2")
    t3 = rope_p.tile([32, n], BF16, tag="t3")
    t4 = rope_p.tile([32, n], BF16, tag="t4")
    nc.vector.tensor_mul(t1, dst[0:32, :], cs[0:32, :])
    nc.vector.tensor_mul(t2, dst[32:64, :], sn[32:64, :])
    nc.vector.tensor_mul(t3, dst[32:64, :], cs[32:64, :])
    nc.vector.tensor_mul(t4, dst[0:32, :], sn[0:32, :])
    nc.vector.tensor_sub(dst[0:32, :], t1, t2)
    nc.vector.tensor_add(dst[32:64, :], t3, t4)


def build_program():
    nc = bacc.Bacc(None, target_bir_lowering=False)
    x_kv = nc.declare_dram_parameter("x_kv", [TKV, W], BF16, isOutput=False)
    wq = nc.declare_dram_parameter("wq", [NH * 128, W], BF16, isOutput=False)
    wk = nc.declare_dram_parameter("wk", [128, W], BF16, isOutput=False)
    wv = nc.declare_dram_parameter("wv", [128, W], BF16, isOutput=False)
    wf = nc.declare_dram_parameter("wf", [2 * NKT * 128, TQ], BF16, isOutput=False)
    bias = nc.declare_dram_parameter("bias", [1, W], F32, isOutput=False)
    cos_t = nc.declare_dram_parameter("cos_t", [64, TKV], BF16, isOutput=False)
    sin_t = nc.declare_dram_parameter("sin_t", [64, TKV], BF16, isOutput=False)
    m0 = nc.declare_dram_parameter("m0", [128, 128], BF16, isOutput=False)
    m8 = nc.declare_dram_parameter("m8", [128, 128], BF16, isOutput=False)
    invc = nc.declare_dram_parameter("invc", [128, NQT], F32, isOutput=False)
    out = nc.declare_dram_parameter("out", [TQ, W], F32, isOutput=True)

    with tile.TileContext(nc) as tc:
        singles_cm = tc.tile_pool(name="singles", bufs=1)
        singles = singles_cm.__enter__()
        ident_f = singles.tile([128, 128], F32)
        make_identity(nc, ident_f)
        ident_b = singles.tile([128, 128], BF16)
        nc.vector.tensor_copy(ident_b, ident_f)
        cos_sb = singles.tile([64, TKV], BF16)
        sin_sb = singles.tile([64, TKV], BF16)
        m0_sb = singles.tile([128, 128], BF16)
        m8_sb = singles.tile([128, 128], BF16)
        invc_sb = singles.tile([128, NQT], F32)

        def load_singles():  # issued after the first x-row DMAs (see below)
            nc.sync.dma_start(out=cos_sb, in_=cos_t[:, :])
            nc.sync.dma_start(out=sin_sb, in_=sin_t[:, :])
            nc.sync.dma_start(out=m0_sb, in_=m0[:, :])
            nc.sync.dma_start(out=m8_sb, in_=m8[:, :])
            nc.sync.dma_start(out=invc_sb, in_=invc[:, :])

        xthp_cm = tc.tile_pool(name="xthp", bufs=NKT)
        xthp = xthp_cm.__enter__()
        kvp_cm = tc.tile_pool(name="kvp", bufs=1)
        kvp = kvp_cm.__enter__()
        xtlp_cm = tc.tile_pool(name="xtlp", bufs=NKT)
        xtlp = xtlp_cm.__enter__()

        xT_lo = []  # xT_lo[kt] = [128 w, 1024 t] bf16 (t in [0,1024))
        xT_hi = []  # t in [1024, 2048)
        for kt in range(NKT):
            xT_lo.append(xtlp.tile([128, TQ], BF16, tag="big", name=f"xtlo{kt}"))
            xT_hi.append(xthp.tile([128, TQ], BF16, tag="xth", name=f"xthi{kt}"))

        # ---- Phase 1+2 interleaved: per 512-token chunk, load+transpose x,
        # then immediately run the K/V projection for that chunk so PE stays
        # busy while the next chunk's x rows stream in.
        kT = kvp.tile([128, TKV], BF16, tag="kT")
        v_aug = []
        for st in range(NST):
            va = kvp.tile([128, 130], BF16, tag=f"vaug{st}", name=f"vaug{st}")
            nc.vector.memset(va[:, 128:129], 1.0)
            v_aug.append(va)

        with tc.tile_pool(name="xrow", bufs=8) as xrow_p, \
             tc.tile_pool(name="xtps", bufs=2, space="PSUM") as xtps, \
             tc.tile_pool(name="wkv", bufs=1) as wkv_p, \
             tc.tile_pool(name="kvps", bufs=2, space="PSUM") as kvps, \
             tc.tile_pool(name="vtmp", bufs=1) as vtmp_p, \
             tc.tile_pool(name="ropet", bufs=2) as rope_p:
            wk_sb = wkv_p.tile([128, W], BF16, tag="wk")
            wv_sb = wkv_p.tile([128, W], BF16, tag="wv")
            vT_tmp = vtmp_p.tile([128, TKV], BF16, tag="vT")

            for g in range(4):  # groups of 4 row-tiles (512 tokens)
                rows = []
                for j in range(4):
                    tt = g * 4 + j
                    r = xrow_p.tile([128, W], BF16, tag="xrow", name=f"xrow{tt}")
                    # split row DMA so transposes of the low half start early
                    nc.sync.dma_start(out=r[:, 0:TQ], in_=x_kv[tt * 128:(tt + 1) * 128, 0:TQ])
                    nc.sync.dma_start(out=r[:, TQ:W], in_=x_kv[tt * 128:(tt + 1) * 128, TQ:W])
                    rows.append(r)
                if g == 0:  # x rows of group 0 go first in the DMA queue
                    nc.sync.dma_start(out=wk_sb, in_=wk[:, :])
                    nc.sync.dma_start(out=wv_sb, in_=wv[:, :])
                    load_singles()
                half, col = divmod(g * 512, TQ)
                dst_l = xT_lo if half == 0 else xT_hi
                for kt in range(NKT):
                    ps = xtps.tile([128, 512], BF16, tag="xt")
                    for j in range(4):
                        nc.tensor.transpose(
                            ps[:, j * 128:(j + 1) * 128],
                            rows[j][:, kt * 128:(kt + 1) * 128], ident_b)
                    nc.vector.tensor_copy(dst_l[kt][:, col:col + 512], ps)
                # K/V projection chunk over the 512 tokens just transposed
                ps_k = kvps.tile([128, 512], F32, tag="pk")
                ps_v = kvps.tile([128, 512], F32, tag="pv")
                for kt in range(NKT):
                    nc.tensor.matmul(ps_k, wk_sb[:, kt * 128:(kt + 1) * 128],
                                     dst_l[kt][:, col:col + 512],
                                     start=(kt == 0), stop=(kt == NKT - 1))
                for kt in range(NKT):
                    nc.tensor.matmul(ps_v, wv_sb[:, kt * 128:(kt + 1) * 128],
                                     dst_l[kt][:, col:col + 512],
                                     start=(kt == 0), stop=(kt == NKT - 1))
                # RoPE on k (rows 0:64), pass rows 64:128
                dst = kT[:, g * 512:(g + 1) * 512]
                nc.vector.tensor_copy(dst, ps_k)
                _rope(nc, dst, cos_sb[:, g * 512:(g + 1) * 512],
                      sin_sb[:, g * 512:(g + 1) * 512], rope_p)
                nc.vector.tensor_copy(vT_tmp[:, g * 512:(g + 1) * 512], ps_v)
                for sj in range(4):
                    st = g * 4 + sj
                    ps_t = kvps.tile([128, 128], BF16, tag="vt")
                    nc.tensor.transpose(ps_t, vT_tmp[:, st * 128:(st + 1) * 128], ident_b)
                    nc.vector.tensor_copy(v_aug[st][:, 0:128], ps_t)
        xtlp_cm.__exit__(None, None, None)

        # ---- Phase 3: per-head attention; encT stays in SBUF ----
        encp_cm = tc.tile_pool(name="encp", bufs=NKT)
        encp = encp_cm.__enter__()
        encT = []
        for h in range(NH):
            encT.append(encp.tile([128, TQ], BF16, tag="enc", name=f"encT{h}"))

        wfp_cm = tc.tile_pool(name="wfp", bufs=2 * NKT)
        wf_p = wfp_cm.__enter__()
        wf_sb = []
        for i in range(2 * NKT):
            wf_sb.append(wf_p.tile([128, TQ], BF16, tag="wf", name=f"wf{i}"))
        b_cm = tc.tile_pool(name="bp", bufs=1)
        b_p = b_cm.__enter__()
        bias_rep = b_p.tile([128, W], F32, tag="bias")
        nc.sync.dma_start(out=bias_rep, in_=bias[:, :].to_broadcast([128, W]))
        # prefetch first half of Wf during attention
        for i in range(NKT):
            nc.sync.dma_start(out=wf_sb[i], in_=wf[i * 128:(i + 1) * 128, :])

        with tc.tile_pool(name="wqp", bufs=2) as wq_p, \
             tc.tile_pool(name="qtp", bufs=2) as qt_p, \
             tc.tile_pool(name="prp", bufs=24) as pr_p, \
             tc.tile_pool(name="ropeq", bufs=2) as ropeq_p, \
             tc.tile_pool(name="encsp", bufs=4) as encs_p, \
             tc.tile_pool(name="dnp", bufs=8) as dn_p, \
             tc.tile_pool(name="qps", bufs=2, space="PSUM") as qps, \
             tc.tile_pool(name="lgps", bufs=3, space="PSUM") as lgps, \
             tc.tile_pool(name="encps", bufs=2, space="PSUM") as encps, \
             tc.tile_pool(name="etps", bufs=1, space="PSUM") as etps:
            for h in range(NH):
                wq_h = wq_p.tile([128, W], BF16, tag="wqh")
                nc.sync.dma_start(out=wq_h, in_=wq[h * 128:(h + 1) * 128, :])
                qT = qt_p.tile([128, TQ], BF16, tag="qT")
                for half in range(2):
                    ps_q = qps.tile([128, 512], F32, tag="q")
                    for kt in range(NKT):
                        nc.tensor.matmul(ps_q, wq_h[:, kt * 128:(kt + 1) * 128],
                                         xT_hi[kt][:, half * 512:(half + 1) * 512],
                                         start=(kt == 0), stop=(kt == NKT - 1))
                    dst = qT[:, half * 512:(half + 1) * 512]
                    nc.vector.tensor_copy(dst, ps_q)
                    _rope(nc, dst,
                          cos_sb[:, TQ + half * 512: TQ + (half + 1) * 512],
                          sin_sb[:, TQ + half * 512: TQ + (half + 1) * 512],
                          ropeq_p)

                if h == 8:  # prefetch second half of Wf
                    for i in range(NKT, 2 * NKT):
                        nc.sync.dma_start(out=wf_sb[i],
                                          in_=wf[i * 128:(i + 1) * 128, :])

                probs = {}  # st -> (qlo, chunks); chunks of <=512 q-cols
                etp_box = [None]

                def do_pv(qt, h=h, probs=probs, etp_box=etp_box):
                    ps_e = encps.tile([128, 129], F32, tag="enc")
                    for d in range(NB):
                        st2 = qt + d
                        qlo2, chunks2 = probs[st2]
                        col = (qt - qlo2) * 128
                        pc2, _ = chunks2[col // 512]
                        off = col % 512
                        nc.tensor.matmul(ps_e, pc2[:, off:off + 128], v_aug[st2][:, 0:129],
                                         start=(d == 0), stop=(d == NB - 1))
                    den = dn_p.tile([128, 1], F32, tag="den")
                    nc.vector.tensor_sub(den, ps_e[:, 128:129], invc_sb[:, qt:qt + 1])
                    rec = dn_p.tile([128, 1], F32, tag="rec")
                    nc.vector.reciprocal(rec, den)
                    enc_s = encs_p.tile([128, 128], BF16, tag="encs")
                    nc.vector.tensor_scalar_mul(enc_s, ps_e[:, 0:128], rec)
                    if qt % 4 == 0:
                        etp_box[0] = etps.tile([128, 512], BF16, tag="et",
                                               name=f"etp{h}_{qt}")
                    nc.tensor.transpose(etp_box[0][:, (qt % 4) * 128:(qt % 4 + 1) * 128],
                                        enc_s, ident_b)
                    if qt % 4 == 3:
                        nc.vector.tensor_copy(
                            encT[h][:, (qt - 3) * 128:(qt + 1) * 128], etp_box[0])

                for st in range(NST):
                    qlo = max(0, st - 8)
                    qhi = min(NQT - 1, st)
                    wst = (qhi - qlo + 1) * 128
                    chunks = []
                    for c0 in range(0, wst, 512):
                        cw = min(512, wst - c0)
                        ps_l = lgps.tile([128, 512], F32, tag="lg")
                        nc.tensor.matmul(ps_l[:, :cw], kT[:, st * 128:(st + 1) * 128],
                                         qT[:, qlo * 128 + c0: qlo * 128 + c0 + cw],
                                         start=True, stop=True)
                        pc = pr_p.tile([128, 512], BF16, tag="pr", name=f"pr{h}_{st}_{c0}")
                        nc.scalar.activation(pc[:, :cw], ps_l[:, :cw],
                                             mybir.ActivationFunctionType.Exp, scale=SCALE)
                        chunks.append((pc, cw))
                    probs[st] = (qlo, chunks)
                    # partial diagonal masks
                    if qhi == st:  # d0 block: cols of qt==st
                        col = (st - qlo) * 128
                        pc, _ = chunks[col // 512]
                        off = col % 512
                        nc.vector.tensor_mul(pc[:, off:off + 128], pc[:, off:off + 128], m0_sb)
                    if qlo == st - 8:  # d8 block: cols of qt==st-8 (first block)
                        pc, _ = chunks[0]
                        nc.vector.tensor_mul(pc[:, 0:128], pc[:, 0:128], m8_sb)
                    # PV deferred one st so exp/mask of the last band block has
                    # a full logits round to complete before PV consumes it
                    if st >= 9:
                        do_pv(st - 9)
                do_pv(7)

        # ---- Phase 4: final projection out = encT.T @ Wf + bias ----
        with tc.tile_pool(name="orow", bufs=2) as orow_p, \
             tc.tile_pool(name="fps", bufs=2, space="PSUM") as fps:
            for dcp in range(2):
                for tt in range(NQT):
                    ps0 = fps.tile([128, 512], F32, tag="f0")
                    ps1 = fps.tile([128, 512], F32, tag="f1")
                    for kt in range(NKT):
                        lhs = encT[kt][:, tt * 128:(tt + 1) * 128]
                        wt = wf_sb[dcp * NKT + kt]
                        nc.tensor.matmul(ps0, lhs, wt[:, 0:512],
                                         start=(kt == 0), stop=(kt == NKT - 1))
                        nc.tensor.matmul(ps1, lhs, wt[:, 512:1024],
                                         start=(kt == 0), stop=(kt == NKT - 1))
                    ot = orow_p.tile([128, TQ], F32, tag="orow")
                    nc.vector.tensor_add(ot[:, 0:512], ps0, bias_rep[:, dcp * TQ:dcp * TQ + 512])
                    nc.sync.dma_start(
                        out=out[tt * 128:(tt + 1) * 128, dcp * TQ:dcp * TQ + 512],
                        in_=ot[:, 0:512])
                    nc.vector.tensor_add(ot[:, 512:1024], ps1,
                                         bias_rep[:, dcp * TQ + 512:(dcp + 1) * TQ])
                    nc.sync.dma_start(
                        out=out[tt * 128:(tt + 1) * 128, dcp * TQ + 512:(dcp + 1) * TQ],
                        in_=ot[:, 512:1024])
        b_cm.__exit__(None, None, None)
        wfp_cm.__exit__(None, None, None)
        encp_cm.__exit__(None, None, None)
        kvp_cm.__exit__(None, None, None)
        xthp_cm.__exit__(None, None, None)
        singles_cm.__exit__(None, None, None)
    nc.finalize()
    return nc


_NC = None


def _get_nc():
    global _NC
    if _NC is None:
        _NC = build_program()
    return _NC


def make_in_maps(x, Wq, Wk, Wv, Wf, bf, segment_pos):
    bf16 = ml_dtypes.bfloat16
    x = np.asarray(x, np.float32)
    r = np.arange(128)
    m0_h = (r[:, None] > r[None, :]).astype(bf16)   # valid jj > r
    m8_h = (r[:, None] <= r[None, :]).astype(bf16)  # valid jj <= r
    inv_ts = (10000.0 ** (-2.0 * np.arange(32, dtype=np.float32) / 64.0))
    wq_r = np.ascontiguousarray(
        np.asarray(Wq, np.float32).reshape(NKT, 128, NH, 128)
        .transpose(2, 1, 0, 3).reshape(NH * 128, W)).astype(bf16)
    wk_r = np.ascontiguousarray(
        np.asarray(Wk, np.float32).reshape(NKT, 128, 128)
        .transpose(1, 0, 2).reshape(128, W)).astype(bf16)
    wv_r = np.ascontiguousarray(
        np.asarray(Wv, np.float32).reshape(NKT, 128, 128)
        .transpose(1, 0, 2).reshape(128, W)).astype(bf16)
    wf_r = np.ascontiguousarray(
        np.asarray(Wf, np.float32).reshape(NKT, 128, 2, TQ)
        .transpose(2, 0, 1, 3).reshape(2 * NKT * 128, TQ)).astype(bf16)
    in_maps = []
    for core in range(8):
        b, qc = core // 4, core % 4
        if qc == 0:
            x_kv = np.concatenate([np.zeros((WIN, W), np.float32), x[b, :TQ]], 0)
            invc_h = np.maximum(0, (WIN - 1) - np.arange(TQ)).astype(np.float32)
        else:
            x_kv = x[b, (qc - 1) * TQ:(qc + 1) * TQ]
            invc_h = np.zeros(TQ, np.float32)
        pos_kv = ((qc - 1) * TQ + np.arange(TKV)).astype(np.float32)
        sinu = pos_kv[None, :] * inv_ts[:, None]
        in_maps.append({
            "x_kv": np.ascontiguousarray(x_kv).astype(bf16),
            "wq": wq_r,
            "wk": wk_r,
            "wv": wv_r,
            "wf": wf_r,
            "bias": np.asarray(bf, np.float32).reshape(1, W),
            "cos_t": np.concatenate([np.cos(sinu)] * 2, 0).astype(bf16),
            "sin_t": np.concatenate([np.sin(sinu)] * 2, 0).astype(bf16),
            "m0": m0_h, "m8": m8_h,
            "invc": invc_h.reshape(NQT, 128).T.copy(),
        })
    return in_maps


def kernel(x, Wq, Wk, Wv, Wf, bf, segment_pos, _trace=False):
    nc = _get_nc()
    in_maps = make_in_maps(x, Wq, Wk, Wv, Wf, bf, segment_pos)
    res = run_bass_kernel_spmd(nc, in_maps, list(range(8)), trace=_trace)
    outs = res.results
    full = np.zeros((B, T, W), np.float32)
    for core in range(8):
        b, qc = core // 4, core % 4
        full[b, qc * TQ:(qc + 1) * TQ] = outs[core]["out"]
    if _trace:
        return full, res
    return full
